# revision 1
# baseline (speedup 1.0000x reference)
"""Gated max/avg 2x2 pooling kernel for Trainium2 (8 NeuronCores, SPMD).

Reference computation (per 2x2 window over [B, H, W, C], stride 2):
    x1 = max(window), x2 = mean(window)
    xs = sum_ij mask[i, j] * window[i, j]   (per channel)
    z  = sigmoid(xs)
    out = z * x1 + (1 - z) * x2

Sharding: pure data-parallel over batch (16 batches -> 2 per core); the
2x2 mask is folded into per-partition scalars computed on the host.

Device layout per core: partition dim = 128 output rows (h); one
macro-tile = (batch, w-quarter) holding even input rows E and odd input
rows O as [128, 4096] f32 tiles (16 KiB contiguous per partition).
Within a tile, free dim = (w_pair 32, even/odd 2, channel 64), so
even/odd w columns are strided sub-APs.

xs is evaluated as a Horner-style chain so each step is one fused DVE
scalar_tensor_tensor op:  t = r_k * t + T_{k+1},  xs = f * t3, with the
terms ordered by ascending |mask| so every ratio r_k has |r_k| <= 1.
The final scale f rides the ACT sigmoid's free affine (sigmoid(f*t3)).
"""

import numpy as np

import concourse.bacc as bacc
import concourse.mybir as mybir
import concourse.tile as tile
from concourse.bass_utils import run_bass_kernel_spmd

F32 = mybir.dt.float32
F16 = mybir.dt.float16

# "f32": exact (rel err ~2.5e-06, ~236 us).  "fp16": intermediates stored
# as float16 to engage the DVE 2x_1p perf mode (rel err ~9e-04, much
# faster).  Inputs/outputs and the final combine stay f32 either way.
PRECISION = "fp16"

B, H, W, C = 16, 256, 256, 64
N_CORES = 8
BPC = B // N_CORES          # batches per core
HO = H // 2                 # 128 output rows = SBUF partitions
NQ = 4                      # w-quarters per row
WQ = W // NQ                # input w per macro-tile (64)

# Set by kernel() when tracing is enabled (env KERNEL_TRACE=1).
LAST_EXEC_NS = None
LAST_RESULTS = None

_PROGRAM_CACHE = {}


def _build_program(bpc, ho, nq, wq, ch, perm=(0, 1, 2, 3), prec="f32"):
    """Build + compile the single-core Bass/Tile program (SPMD-shared).

    perm: order in which the xs Horner chain consumes the window terms
    (Ee, Eo, Oe, Oo); chosen on the host as argsort(|mask|) so every
    chain ratio has magnitude <= 1."""
    from contextlib import ExitStack

    assert ho == 128, "partition dim must be 128"
    fd_in = wq * ch            # free dim of an E/O tile
    wo = wq // 2               # output w per macro-tile
    fd_out = wo * ch           # free dim of output tile

    nc = bacc.Bacc(
        "TRN2",
        target_bir_lowering=False,
        debug=False,
        enable_asserts=True,
        num_devices=N_CORES,
    )

    x = nc.dram_tensor("x", [bpc, ho, 2, nq, fd_in], F32, kind="ExternalInput")
    scal = nc.dram_tensor("scal", [128, 8], F32, kind="ExternalInput")
    out = nc.dram_tensor("out", [bpc, ho, nq, fd_out], F32, kind="ExternalOutput")
    x_ap = x.ap()
    out_ap = out.ap()

    alu = mybir.AluOpType

    with tile.TileContext(nc) as tc, ExitStack() as ctx:
        pool_io = ctx.enter_context(tc.tile_pool(name="io", bufs=2))
        pool_big = ctx.enter_context(tc.tile_pool(name="big", bufs=1))
        pool_tmp = ctx.enter_context(tc.tile_pool(name="tmp", bufs=1))
        pool_t = ctx.enter_context(tc.tile_pool(name="tchain", bufs=2))
        pool_out = ctx.enter_context(tc.tile_pool(name="outp", bufs=2))
        pool_const = ctx.enter_context(tc.tile_pool(name="const", bufs=1))

        scal_t = pool_const.tile([128, 8], F32)
        nc.sync.dma_start(scal_t[:], scal.ap()[:])
        r_aps = [scal_t[:, k : k + 1] for k in range(3)]
        u_aps = [scal_t[:, k : k + 1] for k in range(4)]
        f_ap = scal_t[:, 5:6] if prec == "fp16" else scal_t[:, 3:4]
        zero_ap = scal_t[:, 4:5]  # zeros; avoids a const-table load
        # xs chain scalars are baked for the FIXED term order
        # (Ee, Eo, Oe, Oo): slot k scalar = scal[:, k], final scale
        # rides the sigmoid (scal[:, 3]).  See _mask_scalars().

        def emit_load(b, q, w_lo, w_hi):
            """Stage 1: DMA + ACT casts for one tile (issued one tile
            ahead so ACT never head-of-line blocks on sigmoid(i))."""
            nw = w_hi - w_lo
            fde = nw * 2 * ch
            fdo = nw * ch
            EO = pool_io.tile([128, 2 * fde], F32, tag="EO")
            src = x_ap[b, :, :, q, :].rearrange(
                "p r (w c) -> p r w c", c=2 * ch
            )[:, :, w_lo : w_lo + nw, :]
            nc.sync.dma_start(
                EO[:].rearrange("p (r w c) -> p r w c", r=2, c=2 * ch), src
            )
            h = dict(b=b, q=q, w_lo=w_lo, nw=nw, fde=fde, fdo=fdo)
            if prec == "fp16":
                EOw = pool_io.tile([128, 2 * fde], F16, tag="EO16")
                nc.scalar.copy(EOw[:], EO[:])
                EOsl = EO[:].rearrange(
                    "p (r w e c) -> p r w e c", r=2, e=2, c=ch
                )
                terms_f32 = [
                    EOsl[:, 0, :, 0, :], EOsl[:, 0, :, 1, :],
                    EOsl[:, 1, :, 0, :], EOsl[:, 1, :, 1, :],
                ]
                U = pool_t.tile([128, 3 * fdo], F16, tag="U")
                U4 = U[:].rearrange("p (k w c) -> p k w c", k=3, c=ch)
                for k in range(3):
                    # 3 of 4 scaled casts on ACT; the 4th runs on DVE in
                    # the compute stage (tensor_scalar @4x) to balance
                    # the two engines.
                    nc.scalar.mul(U4[:, k], terms_f32[perm[k]], u_aps[k])
                h["U4"] = U4
            else:
                EOw = EO
            h["EOw"] = EOw
            return h

        def emit_compute(h):
            """Stage 2: all DVE ops + sigmoid + output DMA for one tile."""
            b, q, w_lo, nw = h["b"], h["q"], h["w_lo"], h["nw"]
            fde, fdo, EOw = h["fde"], h["fdo"], h["EOw"]
            wdt = F16 if prec == "fp16" else F32

            def tmp3(tag, pool=pool_tmp, fd=fdo, dt=wdt):
                t = pool.tile([128, fd], dt, tag=tag)
                return t, t[:].rearrange("p (w c) -> p w c", c=ch)

            Ef = EOw[:, 0:fde].rearrange("p (w c) -> p w c", c=ch)
            Of = EOw[:, fde : 2 * fde].rearrange("p (w c) -> p w c", c=ch)

            # xs path
            if prec == "fp16":
                U4 = h["U4"]
                EO4h = EOw[:].rearrange(
                    "p (r w e c) -> p r w e c", r=2, e=2, c=ch
                )
                terms16 = [
                    EO4h[:, 0, :, 0, :], EO4h[:, 0, :, 1, :],
                    EO4h[:, 1, :, 0, :], EO4h[:, 1, :, 1, :],
                ]
                u3, u3v = tmp3("u3")
                nc.vector.tensor_scalar_mul(u3v, terms16[perm[3]], u_aps[3])
                a1, a1v = tmp3("a1", pool_t)
                nc.vector.tensor_add(a1v, U4[:, 0], U4[:, 1])
                a2, a2v = tmp3("a2", pool_t)
                nc.vector.tensor_add(a2v, U4[:, 2], u3v)
                t3, t3v = tmp3("t3", pool_t)
                nc.vector.tensor_add(t3v, a1v, a2v)
            else:
                EO4 = EOw[:].rearrange(
                    "p (r w e c) -> p r w e c", r=2, e=2, c=ch
                )
                terms = [
                    EO4[:, 0, :, 0, :], EO4[:, 0, :, 1, :],
                    EO4[:, 1, :, 0, :], EO4[:, 1, :, 1, :],
                ]
                t1, t1v = tmp3("t", pool_t)
                nc.vector.scalar_tensor_tensor(
                    t1v, terms[perm[0]], r_aps[0], terms[perm[1]],
                    alu.mult, alu.add,
                )
                t2, t2v = tmp3("t", pool_t)
                nc.vector.scalar_tensor_tensor(
                    t2v, t1v, r_aps[1], terms[perm[2]], alu.mult, alu.add
                )
                t3, t3v = tmp3("t", pool_t)
                nc.vector.scalar_tensor_tensor(
                    t3v, t2v, r_aps[2], terms[perm[3]], alu.mult, alu.add
                )
            z, zv = tmp3("z")
            nc.scalar.activation(
                zv,
                t3v,
                mybir.ActivationFunctionType.Sigmoid,
                bias=zero_ap,
                scale=f_ap,
            )

            # max pool: full-width vertical max, then horizontal
            M1, M1v = tmp3("M1", pool_big, fd=fde)
            nc.vector.tensor_max(M1v, Ef, Of)
            M13 = M1[:, 0:fde].rearrange("p (w e c) -> p w e c", e=2, c=ch)
            x1, x1v = tmp3("x1")
            nc.vector.tensor_max(x1v, M13[:, :, 0, :], M13[:, :, 1, :])

            # sum: s = Ee + Eo + Oe + Oo  (x2 = s / 4)
            S1, S1v = tmp3("S1", pool_big, fd=fde)
            nc.vector.tensor_add(S1v, Ef, Of)
            S13 = S1[:, 0:fde].rearrange("p (w e c) -> p w e c", e=2, c=ch)
            s, sv = tmp3("s")
            nc.vector.tensor_add(sv, S13[:, :, 0, :], S13[:, :, 1, :])

            # gating: out = 0.25*s + z*(x1 - 0.25*s)
            d, dv = tmp3("d")
            nc.vector.scalar_tensor_tensor(dv, sv, -0.25, x1v, alu.mult, alu.add)
            g, gv = tmp3("g")
            nc.vector.tensor_mul(gv, zv, dv)
            o, ov = tmp3("o", pool_out, dt=F32)
            nc.vector.scalar_tensor_tensor(ov, sv, 0.25, gv, alu.mult, alu.add)

            dst = out_ap[b, :, q, :].rearrange("p (w c) -> p w c", c=ch)
            nc.sync.dma_start(
                dst[:, w_lo : w_lo + nw, :],
                o[:].rearrange("p (w c) -> p w c", c=ch),
            )

        wo_q = wq // 2  # output w-pairs per quarter
        tiles = []
        for b in range(bpc):
            for q in range(nq):
                if not tiles:
                    # halve the first tile to cut the startup stall
                    tiles.append((b, q, 0, wo_q // 2))
                    tiles.append((b, q, wo_q // 2, wo_q))
                else:
                    tiles.append((b, q, 0, wo_q))
        pending = emit_load(*tiles[0])
        for i in range(len(tiles)):
            nxt = emit_load(*tiles[i + 1]) if i + 1 < len(tiles) else None
            emit_compute(pending)
            pending = nxt

    nc.compile()
    return nc


def _get_program(bpc, ho, nq, wq, ch, perm, prec):
    key = (bpc, ho, nq, wq, ch, perm, prec)
    if key not in _PROGRAM_CACHE:
        _PROGRAM_CACHE[key] = _build_program(bpc, ho, nq, wq, ch, perm, prec)
    return _PROGRAM_CACHE[key]


def _mask_scalars(mask):
    """Chain order + per-partition scalar tensor [128, 8] for the xs chain.

    With terms T ordered by ascending |m| (perm), the Horner chain
    t1 = r0*T[p0] + T[p1]; t2 = r1*t1 + T[p2]; t3 = r2*t2 + T[p3];
    xs = f*t3 uses ratios r_k = m[p_k]/m[p_{k+1}] that all have
    |r_k| <= 1, and f = m[p3] (largest).  A zero denominator implies a
    zero numerator (sorted order), so those ratios are simply 0.
    """
    m = np.asarray(mask, np.float64).reshape(-1)  # m00, m01, m10, m11
    perm = tuple(int(i) for i in np.argsort(np.abs(m), kind="stable"))
    ms = m[list(perm)]
    r = [ms[k] / ms[k + 1] if ms[k + 1] != 0.0 else 0.0 for k in range(3)]
    scal = np.zeros((128, 8), np.float32)
    if PRECISION == "fp16":
        # cols 0-3: u_k = m[perm_k]/f (|u_k| <= 1); col 5: f
        f = ms[3]
        for k in range(4):
            scal[:, k] = ms[k] / f if f != 0.0 else 0.0
        scal[:, 5] = f
    else:
        scal[:, 0] = r[0]
        scal[:, 1] = r[1]
        scal[:, 2] = r[2]
        scal[:, 3] = ms[3]
    return perm, scal


def kernel(x, mask):
    import os

    global LAST_EXEC_NS, LAST_RESULTS

    x = np.asarray(x)
    mask = np.asarray(mask)
    assert x.shape == (B, H, W, C), x.shape
    in_dtype = x.dtype

    perm, scal = _mask_scalars(mask)
    nc = _get_program(BPC, HO, NQ, WQ, C, perm, PRECISION)

    xv = np.ascontiguousarray(x, np.float32).reshape(B, HO, 2, NQ, WQ * C)

    in_maps = [
        {"x": xv[i * BPC : (i + 1) * BPC], "scal": scal} for i in range(N_CORES)
    ]

    trace = os.environ.get("KERNEL_TRACE", "0") == "1"
    res = run_bass_kernel_spmd(
        nc, in_maps, core_ids=list(range(N_CORES)), trace=trace
    )
    LAST_EXEC_NS = res.exec_time_ns
    LAST_RESULTS = res

    parts = [
        r["out"].reshape(BPC, HO, NQ, WQ // 2, C).reshape(BPC, HO, W // 2, C)
        for r in res.results
    ]
    full = np.concatenate(parts, axis=0)
    return full.astype(in_dtype, copy=False)


def _numpy_reference(x, mask):
    xr = x.reshape(x.shape[0], x.shape[1] // 2, 2, x.shape[2] // 2, 2, x.shape[3])
    x1 = xr.max(axis=(2, 4))
    x2 = xr.mean(axis=(2, 4))
    xs = np.einsum("bhiwjc,ij->bhwc", xr, mask)
    z = 1.0 / (1.0 + np.exp(-xs))
    return z * x1 + (1.0 - z) * x2


if __name__ == "__main__":
    # Small-scale CoreSim self-test (no hardware needed).
    from concourse.bass_interp import CoreSim

    rng = np.random.default_rng(0)
    bpc_s, nq_s, wq_s = 1, 1, 8
    h_s, w_s = 256, nq_s * wq_s
    xs_np = rng.standard_normal((bpc_s, h_s, w_s, C)).astype(np.float32)
    mask_np = (rng.standard_normal((2, 2)) * 0.5).astype(np.float32)

    perm_s, scal_s = _mask_scalars(mask_np)
    nc = _build_program(bpc_s, 128, nq_s, wq_s, C, perm_s, PRECISION)
    sim = CoreSim(nc, trace=False)
    sim.tensor("x")[:] = xs_np.reshape(bpc_s, 128, 2, nq_s, wq_s * C)
    sim.tensor("scal")[:] = scal_s
    sim.simulate()
    got = (
        sim.tensor("out")
        .reshape(bpc_s, 128, nq_s, wq_s // 2, C)
        .reshape(bpc_s, 128, w_s // 2, C)
    )
    want = _numpy_reference(xs_np.astype(np.float64), mask_np.astype(np.float64))
    err = np.abs(got - want)
    rel = err.max() / np.abs(want).max()
    print("CoreSim selftest (%s): max abs err" % PRECISION, err.max(), "rel", rel)
    assert rel < (3e-3 if PRECISION == "fp16" else 1e-5), rel
    print("PASS")



# revision 2
# speedup vs baseline: 1.4599x; 1.4599x over previous
"""Gated max/avg 2x2 pooling kernel for Trainium2 (8 NeuronCores, SPMD).

Reference computation (per 2x2 window over [B, H, W, C], stride 2):
    x1 = max(window), x2 = mean(window)
    xs = sum_ij mask[i, j] * window[i, j]   (per channel)
    z  = sigmoid(xs)
    out = z * x1 + (1 - z) * x2

Sharding: pure data-parallel over batch (16 batches -> 2 per core).

Engine split (per 1024-col tile, partition dim = 128 output rows):
  PE    xs path: 4 diagonal-stationary matmuls (u_k * I) accumulating
        sum_k u_k*T_k in PSUM (T_k = the 4 window terms Ee,Eo,Oe,Oo);
        on alternating tiles also the mean path with a 0.25*I stationary.
  ACT   sigmoid straight out of PSUM (free scale f restores mask norm);
        PSUM->SBUF fp16 copy of the mean.
  DVE   max pool (2 tensor_max) + gating (mul + add), all fp16 @2x.
  GPSIMD  d = x1 - s (one tensor_sub).
Inputs are staged to the device as fp16 (host converts), output returns
fp16 and is upcast on the host: HBM traffic drops 2x vs f32.
"""

import numpy as np

import concourse.bacc as bacc
import concourse.mybir as mybir
import concourse.tile as tile
from concourse.bass_utils import run_bass_kernel_spmd

F32 = mybir.dt.float32
F16 = mybir.dt.float16

B, H, W, C = 16, 256, 256, 64
N_CORES = 8
BPC = B // N_CORES          # batches per core
HO = H // 2                 # 128 output rows = SBUF partitions
NQ = 8                      # w-slices (tiles) per row
WQ = 16                     # output w per tile
N = WQ * C                  # 1024 free elems per partition per output tile
FD = 4 * N                  # input tile free dim (r2 * w16 * e2 * c64)
MMCH = 512                  # matmul moving-free chunk (PSUM bank)

# Fraction of tiles whose mean path runs on PE (rest on DVE).
SUM_PE_PERIOD = 2           # every 2nd tile -> alpha = 0.5

LAST_EXEC_NS = None
LAST_RESULTS = None

_PROGRAM_CACHE = {}


def _build_program(bpc, nq, wq, ch):
    from contextlib import ExitStack

    n = wq * ch
    fd = 4 * n
    nch = n // MMCH             # psum chunks per tile

    nc = bacc.Bacc(
        "TRN2",
        target_bir_lowering=False,
        debug=False,
        enable_asserts=True,
        num_devices=N_CORES,
    )

    x = nc.dram_tensor("x", [bpc, HO, nq, fd], F16, kind="ExternalInput")
    wmat = nc.dram_tensor("wmat", [128, 5 * 128], F16, kind="ExternalInput")
    scal = nc.dram_tensor("scal", [128, 8], F32, kind="ExternalInput")
    out = nc.dram_tensor("out", [bpc, HO, nq, n], F16, kind="ExternalOutput")
    x_ap = x.ap()
    out_ap = out.ap()

    with tile.TileContext(nc) as tc, ExitStack() as ctx:
        pio = ctx.enter_context(tc.tile_pool(name="io", bufs=3))
        pbig = ctx.enter_context(tc.tile_pool(name="big", bufs=2))
        psm = ctx.enter_context(tc.tile_pool(name="small", bufs=2))
        pout = ctx.enter_context(tc.tile_pool(name="outp", bufs=2))
        pconst = ctx.enter_context(tc.tile_pool(name="const", bufs=1))
        ppsum = ctx.enter_context(tc.tile_pool(name="acc", bufs=2, space="PSUM"))

        Wt = pconst.tile([128, 5 * 128], F16)
        nc.sync.dma_start(Wt[:], wmat.ap()[:])
        Sc = pconst.tile([128, 8], F32)
        nc.sync.dma_start(Sc[:], scal.ap()[:])
        f_ap = Sc[:, 0:1]
        zero_ap = Sc[:, 1:2]
        q25_ap = Sc[:, 2:3]
        Wd = [Wt[:, k * 128 : (k + 1) * 128] for k in range(5)]

        def emit_load(b, q):
            EO = pio.tile([128, fd], F16, tag="EO")
            nc.sync.dma_start(EO[:], x_ap[b, :, q, :])
            return dict(b=b, q=q, EO=EO)

        def emit_compute(h, sum_on_pe):
            b, q, EO = h["b"], h["q"], h["EO"]
            EOv = EO[:].rearrange("p (r w e c) -> p r w e c", r=2, e=2, c=ch)
            wpc = MMCH // ch    # moving w's per chunk

            def term(k, c0=0, nw=wq):
                r, e = divmod(k, 2)
                return EOv[:, r, c0 : c0 + nw, e, :]

            # xs path on PE: psum[:, chunk] = sum_k u_k * T_k
            pxs = ppsum.tile([128, n], F32, tag="pxs")
            for cH in range(nch):
                for k in range(4):
                    nc.tensor.matmul(
                        pxs[:, cH * MMCH : (cH + 1) * MMCH],
                        Wd[k],
                        term(k, cH * wpc, wpc),
                        start=(k == 0),
                        stop=(k == 3),
                    )
            z = psm.tile([128, n], F16, tag="z")
            nc.scalar.activation(
                z[:],
                pxs[:],
                mybir.ActivationFunctionType.Sigmoid,
                bias=zero_ap,
                scale=f_ap,
            )

            # mean path: s = (Ee+Eo+Oe+Oo)/4
            s025 = psm.tile([128, n], F16, tag="s025")
            if sum_on_pe:
                ps = ppsum.tile([128, n], F32, tag="ps")
                for cH in range(nch):
                    for k in range(4):
                        nc.tensor.matmul(
                            ps[:, cH * MMCH : (cH + 1) * MMCH],
                            Wd[4],
                            term(k, cH * wpc, wpc),
                            start=(k == 0),
                            stop=(k == 3),
                        )
                nc.scalar.copy(s025[:], ps[:])
            else:
                S1 = pbig.tile([128, 2 * n], F16, tag="S1")
                nc.vector.tensor_add(S1[:], EO[:, 0 : 2 * n], EO[:, 2 * n : fd])
                S1v = S1[:].rearrange("p (w e c) -> p w e c", e=2, c=ch)
                s4 = psm.tile([128, n], F16, tag="s4")
                nc.vector.tensor_add(
                    s4[:].rearrange("p (w c) -> p w c", c=ch),
                    S1v[:, :, 0, :],
                    S1v[:, :, 1, :],
                )
                nc.scalar.mul(s025[:], s4[:], q25_ap)

            # max path on DVE
            M1 = pbig.tile([128, 2 * n], F16, tag="M1")
            nc.vector.tensor_max(M1[:], EO[:, 0 : 2 * n], EO[:, 2 * n : fd])
            M1v = M1[:].rearrange("p (w e c) -> p w e c", e=2, c=ch)
            x1 = psm.tile([128, n], F16, tag="x1")
            nc.vector.tensor_max(
                x1[:].rearrange("p (w c) -> p w c", c=ch),
                M1v[:, :, 0, :],
                M1v[:, :, 1, :],
            )

            # gating: out = s + z*(x1 - s)
            d = psm.tile([128, n], F16, tag="d")
            nc.gpsimd.tensor_sub(d[:], x1[:], s025[:])
            g = psm.tile([128, n], F16, tag="g")
            nc.vector.tensor_mul(g[:], z[:], d[:])
            o = pout.tile([128, n], F16, tag="o")
            nc.vector.tensor_add(o[:], s025[:], g[:])
            nc.sync.dma_start(out_ap[b, :, q, :], o[:])

        tiles = [(b, q) for b in range(bpc) for q in range(nq)]
        pending = emit_load(*tiles[0])
        for i in range(len(tiles)):
            nxt = emit_load(*tiles[i + 1]) if i + 1 < len(tiles) else None
            emit_compute(pending, sum_on_pe=(i % SUM_PE_PERIOD == 0))
            pending = nxt

    nc.compile()
    return nc


def _get_program(key):
    if key not in _PROGRAM_CACHE:
        _PROGRAM_CACHE[key] = _build_program(*key)
    return _PROGRAM_CACHE[key]


def _mask_consts(mask):
    """wmat [128, 5*128] f16 (diagonal stationaries u0..u3, 0.25*I) and
    scal [128, 8] f32 (sigmoid scale f, 0, 0.25)."""
    m = np.asarray(mask, np.float64).reshape(-1)  # m00 m01 m10 m11 = Ee Eo Oe Oo
    f = float(m[np.argmax(np.abs(m))])
    if f == 0.0:
        f = 1.0
    u = m / f
    wmat = np.zeros((128, 5 * 128), np.float16)
    idx = np.arange(128)
    for k in range(4):
        wmat[idx, k * 128 + idx] = np.float16(u[k])
    wmat[idx, 4 * 128 + idx] = np.float16(0.25)
    scal = np.zeros((128, 8), np.float32)
    scal[:, 0] = f
    scal[:, 2] = 0.25
    return wmat, scal


def kernel(x, mask):
    import os

    global LAST_EXEC_NS, LAST_RESULTS

    x = np.asarray(x)
    mask = np.asarray(mask)
    assert x.shape == (B, H, W, C), x.shape
    in_dtype = x.dtype

    wmat, scal = _mask_consts(mask)
    nc = _get_program((BPC, NQ, WQ, C))

    # stage as [b, h, q, r, w, e, c] -> fp16
    xs = np.asarray(x, np.float32).reshape(B, HO, 2, NQ, WQ, 2, C)
    xt = xs.transpose(0, 1, 3, 2, 4, 5, 6)
    xv = np.ascontiguousarray(xt).astype(np.float16).reshape(B, HO, NQ, FD)

    in_maps = [
        {"x": xv[i * BPC : (i + 1) * BPC], "wmat": wmat, "scal": scal}
        for i in range(N_CORES)
    ]

    trace = os.environ.get("KERNEL_TRACE", "0") == "1"
    res = run_bass_kernel_spmd(
        nc, in_maps, core_ids=list(range(N_CORES)), trace=trace
    )
    LAST_EXEC_NS = res.exec_time_ns
    LAST_RESULTS = res

    parts = [
        r["out"].reshape(BPC, HO, NQ * WQ, C).astype(np.float32)
        for r in res.results
    ]
    full = np.concatenate(parts, axis=0)
    return full.astype(in_dtype, copy=False)


def _numpy_reference(x, mask):
    xr = x.reshape(x.shape[0], x.shape[1] // 2, 2, x.shape[2] // 2, 2, x.shape[3])
    x1 = xr.max(axis=(2, 4))
    x2 = xr.mean(axis=(2, 4))
    xs = np.einsum("bhiwjc,ij->bhwc", xr, mask)
    z = 1.0 / (1.0 + np.exp(-xs))
    return z * x1 + (1.0 - z) * x2


if __name__ == "__main__":
    # Small-scale CoreSim self-test (no hardware needed).
    from concourse.bass_interp import CoreSim

    rng = np.random.default_rng(0)
    bpc_s, nq_s = 1, 2
    w_s = nq_s * WQ * 2
    xs_np = rng.standard_normal((bpc_s, H, w_s, C)).astype(np.float32)
    mask_np = (rng.standard_normal((2, 2)) * 0.5).astype(np.float32)

    wmat_s, scal_s = _mask_consts(mask_np)
    nc = _build_program(bpc_s, nq_s, WQ, C)
    sim = CoreSim(nc, trace=False)
    xr = xs_np.reshape(bpc_s, HO, 2, nq_s, WQ, 2, C).transpose(0, 1, 3, 2, 4, 5, 6)
    sim.tensor("x")[:] = (
        np.ascontiguousarray(xr).astype(np.float16).reshape(bpc_s, HO, nq_s, FD)
    )
    sim.tensor("wmat")[:] = wmat_s
    sim.tensor("scal")[:] = scal_s
    sim.simulate()
    got = sim.tensor("out").reshape(bpc_s, HO, nq_s * WQ, C).astype(np.float64)
    want = _numpy_reference(xs_np.astype(np.float64), mask_np.astype(np.float64))
    err = np.abs(got - want)
    rel = err.max() / np.abs(want).max()
    print("CoreSim selftest: max abs err", err.max(), "rel", rel)
    assert rel < 5e-3, rel
    print("PASS")


# revision 8
# speedup vs baseline: 1.6348x; 1.1198x over previous
"""Gated max/avg 2x2 pooling kernel for Trainium2 (8 NeuronCores, SPMD).

Reference computation (per 2x2 window over [B, H, W, C], stride 2):
    x1 = max(window), x2 = mean(window)
    xs = sum_ij mask[i, j] * window[i, j]   (per channel)
    z  = sigmoid(xs)
    out = z * x1 + (1 - z) * x2

Sharding: pure data-parallel over batch (16 batches -> 2 per core).

Engine split (per 1024-col tile, partition dim = 128 output rows):
  PE    xs path: 4 diagonal-stationary matmuls (u_k * I) accumulating
        sum_k u_k*T_k in PSUM (T_k = the 4 window terms Ee,Eo,Oe,Oo);
        on alternating tiles also the mean path with a 0.25*I stationary.
  ACT   sigmoid straight out of PSUM (free scale f restores mask norm);
        PSUM->SBUF fp16 copy of the mean.
  DVE   max pool (2 tensor_max) + gating (mul + add), all fp16 @2x.
  GPSIMD  d = x1 - s (one tensor_sub).
Inputs are staged to the device as fp16 (host converts), output returns
fp16 and is upcast on the host: HBM traffic drops 2x vs f32.
"""

import numpy as np

import concourse.bacc as bacc
import concourse.mybir as mybir
import concourse.tile as tile
from concourse.bass_utils import run_bass_kernel_spmd

F32 = mybir.dt.float32
F16 = mybir.dt.float16

B, H, W, C = 16, 256, 256, 64
N_CORES = 8
BPC = B // N_CORES          # batches per core
HO = H // 2                 # 128 output rows = SBUF partitions
NQ = 8                      # w-slices (tiles) per row
WQ = 16                     # output w per tile
N = WQ * C                  # 1024 free elems per partition per output tile
FD = 4 * N                  # input tile free dim (r2 * w16 * e2 * c64)
MMCH = 512                  # matmul moving-free chunk (PSUM bank)

# Tiles whose mean path runs on PE (rest on DVE): every PERIOD-th tile.
SUM_PE_PERIOD = 1           # all tiles on PE

LAST_EXEC_NS = None
LAST_RESULTS = None

_PROGRAM_CACHE = {}


def _build_program(bpc, nq, wq, ch):
    from contextlib import ExitStack

    n = wq * ch
    fd = 4 * n
    nch = n // MMCH             # psum chunks per tile

    nc = bacc.Bacc(
        "TRN2",
        target_bir_lowering=False,
        debug=False,
        enable_asserts=True,
        num_devices=N_CORES,
    )

    x = nc.dram_tensor("x", [bpc, HO, nq, fd], F16, kind="ExternalInput")
    wmat = nc.dram_tensor("wmat", [128, 5 * 128], F16, kind="ExternalInput")
    scal = nc.dram_tensor("scal", [128, 8], F32, kind="ExternalInput")
    out = nc.dram_tensor("out", [bpc, HO, nq, n], F16, kind="ExternalOutput")
    x_ap = x.ap()
    out_ap = out.ap()

    with tile.TileContext(nc) as tc, ExitStack() as ctx:
        pio = ctx.enter_context(tc.tile_pool(name="io", bufs=4))
        pbig = ctx.enter_context(tc.tile_pool(name="big", bufs=2))
        psm = ctx.enter_context(tc.tile_pool(name="small", bufs=2))
        pout = ctx.enter_context(tc.tile_pool(name="outp", bufs=2))
        pconst = ctx.enter_context(tc.tile_pool(name="const", bufs=1))
        ppsum = ctx.enter_context(tc.tile_pool(name="acc", bufs=2, space="PSUM"))

        Wt = pconst.tile([128, 5 * 128], F16)
        nc.sync.dma_start(Wt[:], wmat.ap()[:])
        Sc = pconst.tile([128, 8], F32)
        nc.sync.dma_start(Sc[:], scal.ap()[:])
        f_ap = Sc[:, 0:1]
        zero_ap = Sc[:, 1:2]
        q25_ap = Sc[:, 2:3]
        Wd = [Wt[:, k * 128 : (k + 1) * 128] for k in range(5)]

        def emit_load(b, q):
            EO = pio.tile([128, fd], F16, tag="EO")
            nc.sync.dma_start(EO[:], x_ap[b, :, q, :])
            return dict(b=b, q=q, EO=EO)

        def emit_compute(h, sum_on_pe):
            b, q, EO = h["b"], h["q"], h["EO"]
            EOv = EO[:].rearrange("p (r w e c) -> p r w e c", r=2, e=2, c=ch)
            wpc = MMCH // ch    # moving w's per chunk

            def term(k, c0=0, nw=wq):
                r, e = divmod(k, 2)
                return EOv[:, r, c0 : c0 + nw, e, :]

            # mean path first (frees ACT to copy s while xs still runs):
            # s = (Ee+Eo+Oe+Oo)/4 via PE with 0.25*I stationary
            s025 = psm.tile([128, n], F16, tag="s025")
            if sum_on_pe:
                ps = ppsum.tile([128, n], F32, tag="ps")
                for cH in range(nch):
                    for k in range(4):
                        nc.tensor.matmul(
                            ps[:, cH * MMCH : (cH + 1) * MMCH],
                            Wd[4],
                            term(k, cH * wpc, wpc),
                            start=(k == 0),
                            stop=(k == 3),
                        )
                nc.scalar.copy(s025[:], ps[:])
            else:
                S1 = pbig.tile([128, 2 * n], F16, tag="S1")
                nc.vector.tensor_add(S1[:], EO[:, 0 : 2 * n], EO[:, 2 * n : fd])
                S1v = S1[:].rearrange("p (w e c) -> p w e c", e=2, c=ch)
                s4 = psm.tile([128, n], F16, tag="s4")
                nc.vector.tensor_add(
                    s4[:].rearrange("p (w c) -> p w c", c=ch),
                    S1v[:, :, 0, :],
                    S1v[:, :, 1, :],
                )
                nc.scalar.mul(s025[:], s4[:], q25_ap)

            # xs path on PE: psum[:, chunk] = sum_k u_k * T_k
            pxs = ppsum.tile([128, n], F32, tag="pxs")
            for cH in range(nch):
                for k in range(4):
                    nc.tensor.matmul(
                        pxs[:, cH * MMCH : (cH + 1) * MMCH],
                        Wd[k],
                        term(k, cH * wpc, wpc),
                        start=(k == 0),
                        stop=(k == 3),
                    )
            z = psm.tile([128, n], F16, tag="z")
            nc.scalar.activation(
                z[:],
                pxs[:],
                mybir.ActivationFunctionType.Sigmoid,
                bias=zero_ap,
                scale=f_ap,
            )

            # max path on DVE
            M1 = pbig.tile([128, 2 * n], F16, tag="M1")
            nc.vector.tensor_max(M1[:], EO[:, 0 : 2 * n], EO[:, 2 * n : fd])
            M1v = M1[:].rearrange("p (w e c) -> p w e c", e=2, c=ch)
            x1 = psm.tile([128, n], F16, tag="x1")
            nc.vector.tensor_max(
                x1[:].rearrange("p (w c) -> p w c", c=ch),
                M1v[:, :, 0, :],
                M1v[:, :, 1, :],
            )

            # gating: out = s + z*(x1 - s); the sub rides on GPSIMD
            d = psm.tile([128, n], F16, tag="d")
            nc.gpsimd.tensor_sub(d[:], x1[:], s025[:])
            g = psm.tile([128, n], F16, tag="g")
            nc.vector.tensor_mul(g[:], z[:], d[:])
            o = pout.tile([128, n], F16, tag="o")
            nc.vector.tensor_add(o[:], s025[:], g[:])
            nc.sync.dma_start(out_ap[b, :, q, :], o[:])

        tiles = [(b, q) for b in range(bpc) for q in range(nq)]
        AHEAD = 2
        pending = [emit_load(*tiles[i]) for i in range(min(AHEAD, len(tiles)))]
        for i in range(len(tiles)):
            if i + AHEAD < len(tiles):
                pending.append(emit_load(*tiles[i + AHEAD]))
            emit_compute(pending.pop(0), sum_on_pe=(i % SUM_PE_PERIOD == 0))

    nc.compile()
    return nc


def _get_program(key):
    if key not in _PROGRAM_CACHE:
        _PROGRAM_CACHE[key] = _build_program(*key)
    return _PROGRAM_CACHE[key]


def _mask_consts(mask):
    """wmat [128, 5*128] f16 (diagonal stationaries u0..u3, 0.25*I) and
    scal [128, 8] f32 (sigmoid scale f, 0, 0.25)."""
    m = np.asarray(mask, np.float64).reshape(-1)  # m00 m01 m10 m11 = Ee Eo Oe Oo
    f = float(m[np.argmax(np.abs(m))])
    if f == 0.0:
        f = 1.0
    u = m / f
    wmat = np.zeros((128, 5 * 128), np.float16)
    idx = np.arange(128)
    for k in range(4):
        wmat[idx, k * 128 + idx] = np.float16(u[k])
    wmat[idx, 4 * 128 + idx] = np.float16(0.25)
    scal = np.zeros((128, 8), np.float32)
    scal[:, 0] = f
    scal[:, 2] = 0.25
    return wmat, scal


def kernel(x, mask):
    import os

    global LAST_EXEC_NS, LAST_RESULTS

    x = np.asarray(x)
    mask = np.asarray(mask)
    assert x.shape == (B, H, W, C), x.shape
    in_dtype = x.dtype

    wmat, scal = _mask_consts(mask)
    nc = _get_program((BPC, NQ, WQ, C))

    # stage as [b, h, q, r, w, e, c] -> fp16
    xs = np.asarray(x, np.float32).reshape(B, HO, 2, NQ, WQ, 2, C)
    xt = xs.transpose(0, 1, 3, 2, 4, 5, 6)
    xv = np.ascontiguousarray(xt).astype(np.float16).reshape(B, HO, NQ, FD)

    in_maps = [
        {"x": xv[i * BPC : (i + 1) * BPC], "wmat": wmat, "scal": scal}
        for i in range(N_CORES)
    ]

    trace = os.environ.get("KERNEL_TRACE", "0") == "1"
    res = run_bass_kernel_spmd(
        nc, in_maps, core_ids=list(range(N_CORES)), trace=trace
    )
    LAST_EXEC_NS = res.exec_time_ns
    LAST_RESULTS = res

    parts = [
        r["out"].reshape(BPC, HO, NQ * WQ, C).astype(np.float32)
        for r in res.results
    ]
    full = np.concatenate(parts, axis=0)
    return full.astype(in_dtype, copy=False)


def _numpy_reference(x, mask):
    xr = x.reshape(x.shape[0], x.shape[1] // 2, 2, x.shape[2] // 2, 2, x.shape[3])
    x1 = xr.max(axis=(2, 4))
    x2 = xr.mean(axis=(2, 4))
    xs = np.einsum("bhiwjc,ij->bhwc", xr, mask)
    z = 1.0 / (1.0 + np.exp(-xs))
    return z * x1 + (1.0 - z) * x2


if __name__ == "__main__":
    # Small-scale CoreSim self-test (no hardware needed).
    from concourse.bass_interp import CoreSim

    rng = np.random.default_rng(0)
    bpc_s, nq_s = 1, 2
    w_s = nq_s * WQ * 2
    xs_np = rng.standard_normal((bpc_s, H, w_s, C)).astype(np.float32)
    mask_np = (rng.standard_normal((2, 2)) * 0.5).astype(np.float32)

    wmat_s, scal_s = _mask_consts(mask_np)
    nc = _build_program(bpc_s, nq_s, WQ, C)
    sim = CoreSim(nc, trace=False)
    xr = xs_np.reshape(bpc_s, HO, 2, nq_s, WQ, 2, C).transpose(0, 1, 3, 2, 4, 5, 6)
    sim.tensor("x")[:] = (
        np.ascontiguousarray(xr).astype(np.float16).reshape(bpc_s, HO, nq_s, FD)
    )
    sim.tensor("wmat")[:] = wmat_s
    sim.tensor("scal")[:] = scal_s
    sim.simulate()
    got = sim.tensor("out").reshape(bpc_s, HO, nq_s * WQ, C).astype(np.float64)
    want = _numpy_reference(xs_np.astype(np.float64), mask_np.astype(np.float64))
    err = np.abs(got - want)
    rel = err.max() / np.abs(want).max()
    print("CoreSim selftest: max abs err", err.max(), "rel", rel)
    assert rel < 5e-3, rel
    print("PASS")
